# revision 1
# baseline (speedup 1.0000x reference)
"""Multi-head sigmoid self-attention on 8 Trainium2 NeuronCores.

Sharding: pure data parallel — batch (8) split one element per core.
Per core (feature-major "T" = transposed layouts, all matmuls bf16):

  q^T = Wq @ x^T, k^T = Wk @ x^T   (feature-major)
  v   = x @ Wv^T                   (token-major)
  S^T = sigmoid((k_h q_h^T)/sqrt(F) + bias)  per head; two heads of a
        pair computed concurrently on the PE (row-packed K=64)
  attn^T = v_h^T @ S_h^T           (col-packed pairs of heads)
  o   = attn @ Wo^T + bo           (token-major)

Emission is software-pipelined so the Scalar engine (sigmoid — the
second-busiest engine) starts within a few microseconds and is never
starved: scores for iteration i are emitted before attention matmuls of
iteration i-1, with projection work spread between iterations as PE
filler. Host-side numpy does all transposes/re-striping and the bf16
cast, so every device DMA is a contiguous copy.
"""

import os
import sys

import numpy as np

if "/opt/trn_rl_repo" not in sys.path:
    sys.path.insert(0, "/opt/trn_rl_repo")

P = 128
F = 768
N = 1024
H = 12
HD = 64
KO = F // P      # 6 feature stripes
NT = N // P      # 8 token tiles
CH = 2           # 512-token chunks
CW = N // CH     # 512
HP = H // 2      # 6 head pairs
FW = F // 2      # 384 (psum-sized feature chunk)
SCALE = 1.0 / float(np.sqrt(np.float32(F)))

_CACHE = {}

LAST_EXEC_NS = None


def _build():
    import concourse.mybir as mybir
    import concourse.tile as tile
    from concourse import bacc

    f32 = mybir.dt.float32
    bf16 = mybir.dt.bfloat16
    ADD = mybir.AluOpType.add
    SIG = mybir.ActivationFunctionType.Sigmoid

    nc = bacc.Bacc("TRN2", target_bir_lowering=False, debug=False)

    xT_d = nc.dram_tensor("xT", [P, KO, N], bf16, kind="ExternalInput").ap()
    wq_d = nc.dram_tensor("wqT", [P, KO, F], bf16, kind="ExternalInput").ap()
    wk_d = nc.dram_tensor("wkT", [P, KO, F], bf16, kind="ExternalInput").ap()
    wv_d = nc.dram_tensor("wvT", [P, KO, F], bf16, kind="ExternalInput").ap()
    wo_d = nc.dram_tensor("woT", [P, KO, F], bf16, kind="ExternalInput").ap()
    bq_d = nc.dram_tensor("bqs", [P, KO], f32, kind="ExternalInput").ap()
    bk_d = nc.dram_tensor("bks", [P, KO], f32, kind="ExternalInput").ap()
    bv_d = nc.dram_tensor("bvr", [P, F], f32, kind="ExternalInput").ap()
    bo_d = nc.dram_tensor("bor", [P, F], f32, kind="ExternalInput").ap()
    bi_d = nc.dram_tensor("bir", [P, 1], f32, kind="ExternalInput").ap()
    o_d = nc.dram_tensor("o", [N, F], f32, kind="ExternalOutput").ap()

    with tile.TileContext(nc) as tc:
        with (
            tc.tile_pool(name="sb", bufs=1) as sb,
            tc.tile_pool(name="ps", bufs=1, space="PSUM") as psp,
        ):
            # ---- persistent SBUF tensors -------------------------------
            xT = sb.tile([P, KO, N], bf16, tag="xT")
            wq = sb.tile([P, KO, F], bf16, tag="wq")
            wk = sb.tile([P, KO, F], bf16, tag="wk")
            wv = sb.tile([P, KO, F], bf16, tag="wv")
            wo = sb.tile([P, KO, F], bf16, tag="wo")
            qT = sb.tile([P, KO, N], bf16, tag="qT")
            kT = sb.tile([P, KO, N], bf16, tag="kT")
            v_sb = sb.tile([P, NT, F], bf16, tag="v")
            attnT = sb.tile([P, KO, N], bf16, tag="attnT")
            bqs = sb.tile([P, KO], f32, tag="bqs")
            bks = sb.tile([P, KO], f32, tag="bks")
            bvr = sb.tile([P, F], f32, tag="bvr")
            bor = sb.tile([P, F], f32, tag="bor")
            bir = sb.tile([P, 1], f32, tag="bir")

            # ---- emission helpers --------------------------------------
            def gen_qk_stripe(mo):
                """Generator: q^T/k^T projections for feature stripe mo,
                yielding after every ~2 matmuls so the emission can be
                interleaved between attention steps. Both token chunks
                accumulate side by side (one weight load per two matmuls).
                k before q: scores need the full kT stripe but only one
                qT chunk."""
                for w_sb, bst, dst in ((wk, bks, kT), (wq, bqs, qT)):
                    ps = [psp.tile([P, CW], f32, tag="pp", bufs=3,
                                   name="ps_qk") for _ in range(CH)]
                    for ko in range(KO):
                        for ch in range(CH):
                            nc.tensor.matmul(
                                ps[ch][:],
                                w_sb[:, ko, mo * P:(mo + 1) * P],
                                xT[:, ko, ch * CW:(ch + 1) * CW],
                                start=(ko == 0), stop=(ko == KO - 1),
                            )
                        yield
                    for ch in range(CH):
                        nc.vector.tensor_tensor(
                            dst[:, mo, ch * CW:(ch + 1) * CW], ps[ch][:],
                            bst[:, mo:mo + 1].to_broadcast([P, CW]), ADD,
                        )
                    yield

            def gen_v(trange, j):
                """Generator: v projection (token-major) for token tiles in
                trange, feature chunk j; yields every ~2 matmuls."""
                for t in trange:
                    ps = psp.tile([P, CW], f32, tag="pp", bufs=3, name="ps_v")
                    psv = ps[:, 0:FW]
                    for ko in range(KO):
                        nc.tensor.matmul(
                            psv,
                            xT[:, ko, t * P:(t + 1) * P],
                            wv[:, ko, j * FW:(j + 1) * FW],
                            start=(ko == 0), stop=(ko == KO - 1),
                        )
                        if ko % 2 == 1:
                            yield
                    nc.vector.tensor_tensor(
                        v_sb[:, t, j * FW:(j + 1) * FW], psv,
                        bvr[:, j * FW:(j + 1) * FW], ADD,
                    )
                    yield

            def run_gen(g):
                for _ in g:
                    pass

            def emit_scores(ch, hp):
                """S^T for both heads of pair hp, query chunk ch.
                Returns the 8 sigmoid output tiles (one per key tile)."""
                qsl = slice(ch * CW, (ch + 1) * CW)
                sts = []
                for kt in range(NT):
                    ksl = slice(kt * P, (kt + 1) * P)
                    sc = psp.tile([P, 2, CW], f32, tag="sc", bufs=2,
                                  name="sc")
                    nc.tensor.matmul(sc[:, 0, :], kT[0:64, hp, ksl],
                                     qT[0:64, hp, qsl],
                                     start=True, stop=True)
                    nc.tensor.matmul(sc[:, 1, :], kT[64:128, hp, ksl],
                                     qT[64:128, hp, qsl],
                                     start=True, stop=True)
                    st = sb.tile([P, 2, CW], bf16, tag="st", bufs=18,
                                 name="st")
                    nc.scalar.activation(st[:], sc[:], SIG,
                                         bias=bir[:, 0:1], scale=SCALE)
                    sts.append(st)
                return sts

            def emit_attn(ch, hp, sts):
                """attn^T accumulation for head pair hp over the 8 key
                tiles, then copy out to attnT."""
                qsl = slice(ch * CW, (ch + 1) * CW)
                at = psp.tile([P, CW], f32, tag="at", bufs=1, name="at")
                for kt in range(NT):
                    st = sts[kt]
                    nc.tensor.matmul(at[0:64, :],
                                     v_sb[:, kt, hp * P:hp * P + HD],
                                     st[:, 0, :],
                                     start=(kt == 0), stop=(kt == NT - 1))
                    nc.tensor.matmul(at[64:128, :],
                                     v_sb[:, kt, hp * P + HD:(hp + 1) * P],
                                     st[:, 1, :],
                                     start=(kt == 0), stop=(kt == NT - 1))
                nc.vector.tensor_copy(attnT[:, hp, qsl], at[:])

            def emit_oproj(ch):
                """output projection for the 4 token tiles of chunk ch.
                Both feature chunks accumulate side by side so each attnT
                tile (the stationary operand) is loaded once for two
                matmuls."""
                for tt in range(4):
                    tg = ch * 4 + tt
                    op = sb.tile([P, F], f32, tag="osb", bufs=3, name="osb")
                    ps = [psp.tile([P, CW], f32, tag="pp", bufs=3,
                                   name="ps_o") for _ in range(2)]
                    for ko in range(KO):
                        for j in range(2):
                            nc.tensor.matmul(
                                ps[j][:, 0:FW],
                                attnT[:, ko, tg * P:(tg + 1) * P],
                                wo[:, ko, j * FW:(j + 1) * FW],
                                start=(ko == 0), stop=(ko == KO - 1),
                            )
                    for j in range(2):
                        nc.vector.tensor_tensor(
                            op[:, j * FW:(j + 1) * FW], ps[j][:, 0:FW],
                            bor[:, j * FW:(j + 1) * FW], ADD,
                        )
                    nc.sync.dma_start(o_d[tg * P:(tg + 1) * P, :], op[:])

            # ---- software-pipelined emission ---------------------------
            # Emission order IS program order in Tile: every producer must
            # be emitted before its consumers.  Attention consumption lags
            # score production by TWO iterations so attn matmuls only read
            # sigmoids finished long ago (no PE stalls on the Scalar
            # engine); projection work for later stripes fills the PE
            # between iterations.  Deadlines: qk stripe s before
            # scores(.,s) at iteration 2s; v chunk j before the first
            # attn that reads it (attn(0,0) at it2 needs j=0, attn(0,3)
            # at it8 needs j=1).
            nc.sync.dma_start(bir[:], bi_d)
            for ko in range(KO):
                nc.sync.dma_start(xT[:, ko, :], xT_d[:, ko, :])
                nc.gpsimd.dma_start(wk[:, ko, :], wk_d[:, ko, :])
            nc.sync.dma_start(bks[:], bk_d)
            nc.sync.dma_start(bqs[:], bq_d)
            for ko in range(KO):
                nc.gpsimd.dma_start(wq[:, ko, :], wq_d[:, ko, :])
            run_gen(gen_qk_stripe(0))
            seq = [(ch, hp) for hp in range(HP) for ch in range(CH)]
            for ko in range(KO):
                nc.sync.dma_start(wv[:, ko, :], wv_d[:, ko, :])
            nc.sync.dma_start(bvr[:], bv_d)
            for ko in range(KO):
                nc.sync.dma_start(wo[:, ko, :], wo_d[:, ko, :])
            nc.sync.dma_start(bor[:], bo_d)

            filler = {
                1: lambda: run_gen(gen_qk_stripe(1)),
                2: lambda: run_gen(gen_qk_stripe(2)),
                3: lambda: run_gen(gen_v(range(0, 4), 1)),
                4: lambda: run_gen(gen_qk_stripe(3)),
                5: lambda: run_gen(gen_v(range(4, 8), 1)),
                6: lambda: run_gen(gen_qk_stripe(4)),
                7: lambda: run_gen(gen_qk_stripe(5)),
            }
            pending = []
            for it, (ch, hp) in enumerate(seq):
                if it == 0:
                    pending.append(seq[0] + (emit_scores(*seq[0]),))
                    run_gen(gen_v(range(0, 8), 0))
                    continue
                pending.append((ch, hp, emit_scores(ch, hp)))
                if len(pending) > 1:
                    done = pending.pop(0)
                    emit_attn(*done)
                    if done[0] == 0 and done[1] == HP - 1:
                        emit_oproj(0)
                if it in filler:
                    filler[it]()
            for done in pending:
                emit_attn(*done)
                if done[0] == 0 and done[1] == HP - 1:
                    emit_oproj(0)
            emit_oproj(1)

    nc.compile()
    return nc


def _bf16(a):
    import ml_dtypes
    return np.ascontiguousarray(a).astype(ml_dtypes.bfloat16)


def _prep_w(W):
    W = np.asarray(W, dtype=np.float32)
    return _bf16(W.T.reshape(KO, P, F).transpose(1, 0, 2))


def kernel(x, bias, Wq, bq, Wk, bk, Wv, bv, Wo, bo):
    global LAST_EXEC_NS
    from concourse import bass_utils

    if "nc" not in _CACHE:
        _CACHE["nc"] = _build()
    nc = _CACHE["nc"]

    x = np.asarray(x, dtype=np.float32)
    shared = {
        "wqT": _prep_w(Wq),
        "wkT": _prep_w(Wk),
        "wvT": _prep_w(Wv),
        "woT": _prep_w(Wo),
        "bqs": np.ascontiguousarray(
            np.asarray(bq, np.float32).reshape(KO, P).T),
        "bks": np.ascontiguousarray(
            np.asarray(bk, np.float32).reshape(KO, P).T),
        "bvr": np.ascontiguousarray(
            np.broadcast_to(np.asarray(bv, np.float32), (P, F))),
        "bor": np.ascontiguousarray(
            np.broadcast_to(np.asarray(bo, np.float32), (P, F))),
        "bir": np.full((P, 1), np.float32(np.asarray(bias)),
                       dtype=np.float32),
    }
    in_maps = []
    for b in range(x.shape[0]):
        m = dict(shared)
        m["xT"] = _bf16(x[b].T.reshape(KO, P, N).transpose(1, 0, 2))
        in_maps.append(m)

    trace = bool(os.environ.get("KERNEL_TRACE"))
    if trace:
        try:
            import ntff_hook
            ntff_hook.install()
        except Exception:
            trace = False

    res = bass_utils.run_bass_kernel_spmd(
        nc, in_maps, core_ids=list(range(len(in_maps))), trace=trace)
    LAST_EXEC_NS = res.exec_time_ns
    return np.stack([r["o"] for r in res.results]).astype(np.float32)

